# revision 23
# baseline (speedup 1.0000x reference)
"""MultiHeadSelfAttention Trainium2 kernel (8-core SPMD).

Sharding: batch B=2 x head-groups 4 (4 heads each) -> 8 cores.
Core c: batch b=c//4, head group g=c%4 (heads 4g..4g+4).

Per-core device program:
  1. fp8(e4m3) DoubleRow projections from pre-transposed xT [D, L]
     (host ships x*16, Wq/Wk*256, Wv*16; kd-pairs contract 256 rows/pass):
       QT, KT: [dk=256, L] bf16 (dk on partitions)
       V:      [L, 4*80] fp8, ones column per head -> softmax denominator
  2. Attention per (head-pair hp, q-chunk qc of 512):
       scoresT [128 kpos, 1024] = KT_h.T @ QT_h  (K=64 row-tiled pairs run
       concurrently on the PE array)
       exp -> e4m3, split across engines: ScalarE native Exp (10/16 ktiles)
       + VectorE Schraudolph bit-trick exp (6/16): int8 = s*A + B bitcast
       po_h [65, 512] += V.T @ expT via fp8 DoubleRow over ktile pairs
       (row 64 = softmax denominator, shipped RAW in bf16)
  3. AllToAll (8 ranks, bf16): shard p = q-slice (p%4) of unnormalized oT +
     raw-denominator rows 128/129 -> receiver keeps only its 4 same-batch
     peer blocks (runtime cc_rank offset into cc_out), turns denominators
     into reciprocals with a 2-op uint16 magic trick (bits(1/x) ~= K-bits(x)),
     normalizes og.
  4. Out-projection over the 4 same-batch blocks only (no masked waste),
     residual add, LayerNorm -> out [L/4, D] f32.
"""

import sys

sys.path.insert(0, "/opt/trn_rl_repo")

import numpy as np
import ml_dtypes

import concourse.bass as bass
import concourse.bacc as bacc
import concourse.tile as tile
from concourse import mybir
from concourse import bass_utils
import bass_rust

BF16 = mybir.dt.bfloat16
F32 = mybir.dt.float32
F8 = mybir.dt.float8e4
I8 = mybir.dt.int8
# Schraudolph exp->e4m3-bits on the DVE: bits = round(s_raw*A + B)
# (A = 8*log2(e) * 0.125 score scale; B tuned for zero mean error)
SCHRA_A = 1.4426950408889634 / 16777216.0
SCHRA_B = 55.55
AF = mybir.ActivationFunctionType

_PATCHED = False

ALL_RANKS = [[0, 1, 2, 3, 4, 5, 6, 7]]


def _patch_tile_drain():
    """The installed walrus rejects >1 sem wait on a Drain instruction; split
    the TileContext tail-drain waits across multiple drains."""
    global _PATCHED
    if _PATCHED:
        return
    _PATCHED = True

    def _patched(self, tick_clock, wait_clock):
        from concourse.vector_clock import ScopedClock

        probe = self.nc.sync.drain()
        wait_clock.add_sem_waits(
            probe.ins, ScopedClock({None: tick_clock.global_clock})
        )
        si = probe.ins.sync_info
        waits = list(si.on_wait or []) if si is not None else []
        if len(waits) > 1:
            si.on_wait = [waits[0]]
            for w in waits[1:]:
                d2 = self.nc.sync.drain()
                si2 = d2.ins.sync_info
                if si2 is None:
                    d2.ins.sync_info = bass_rust.SyncInfo(on_wait=[w], on_update=[])
                else:
                    si2.on_wait = [w]
        self.nc.all_engine_barrier()
        assert self.sems is not None
        popped = self.nc._tile_sem_poison_stack.pop()
        assert popped is self._sem_poison
        self.nc.clear_and_free_semaphores(list(self.sems.allocated().values()))
        self.nc.all_engine_barrier()

    tile.TileContext._drain_and_barrier = _patched


def build_nc(L=2048, D=1024, eps=1e-6, trivial_gamma=False, trivial_beta=False):
    """Build the SPMD per-core program. Heads per core = 4, DK=DV=64."""
    _patch_tile_drain()
    HC = 4            # heads per core
    DKV = 64
    PD = HC * DKV     # 256: projected dims per core
    KD = D // 128     # contraction tiles over D
    LT = L // 128     # k-position tiles
    QCN = L // 512    # q chunks of 512
    QS = L // 4       # per-core output rows (q-slice)
    QT = QS // 128    # out row tiles
    HB = 8            # wo row blocks: 4 same-batch peers x 2 head-pairs
    SUB = max(1, D // 512)    # bn_stats subgroups

    nc = bacc.Bacc(num_devices=8, debug=False)

    xT_d = nc.dram_tensor("xT", [D, L], F8, kind="ExternalInput")
    xres_d = nc.dram_tensor("xres", [QS, D], F32, kind="ExternalInput")
    wq_d = nc.dram_tensor("wq", [D, PD], F8, kind="ExternalInput")
    wk_d = nc.dram_tensor("wk", [D, PD], F8, kind="ExternalInput")
    wv_d = nc.dram_tensor("wv", [D, PD], F8, kind="ExternalInput")
    wo_d = nc.dram_tensor("wo", [HB * 128, D], F8, kind="ExternalInput")
    gamma_d = nc.dram_tensor("gamma", [1, D], F32, kind="ExternalInput")
    beta_d = nc.dram_tensor("beta", [1, D], F32, kind="ExternalInput")
    out_d = nc.dram_tensor("out", [QS, D], BF16, kind="ExternalOutput")

    with tile.TileContext(nc) as tc:
        with (
            tc.tile_pool(name="singles", bufs=1) as singles,
            tc.tile_pool(name="work", bufs=3) as work,
            tc.tile_pool(name="small", bufs=4) as small,
            tc.tile_pool(name="psum", bufs=1, space="PSUM") as psum,
            tc.tile_pool(name="dram", bufs=1, space="DRAM") as dram,
        ):
            # ---------- load inputs ----------
            xT_sb = singles.tile([128, KD, L], F8)
            wq_sb = singles.tile([128, KD, PD], F8)
            wk_sb = singles.tile([128, KD, PD], F8)
            wv_sb = singles.tile([128, KD, PD], F8)
            for eng, w_sb, w_d in (
                (nc.scalar, wk_sb, wk_d),
                (nc.scalar, wq_sb, wq_d),
                (nc.gpsimd, wv_sb, wv_d),
            ):
                eng.dma_start(
                    out=w_sb[:, :, :], in_=w_d.ap().rearrange("(t p) m -> p t m", p=128)
                )
            xT_r = xT_d.ap().rearrange("(t p) l -> p t l", p=128)
            for t in range(KD):
                nc.sync.dma_start(out=xT_sb[:, t, :], in_=xT_r[:, t, :])
            wo_sb = singles.tile([128, HB, D], F8)
            xres_sb = singles.tile([128, QT, D], F32)
            gb_sb = singles.tile([128, D], F32)
            bb_sb = singles.tile([128, D], F32)
            # emitted after xT on the same queue: ring FIFO keeps these from
            # stealing HBM bandwidth ahead of the xT critical path
            wo_r = wo_d.ap().rearrange("(t p) n -> p t n", p=128)
            for t in range(HB):
                nc.sync.dma_start(out=wo_sb[:, t, :], in_=wo_r[:, t, :])
            nc.sync.dma_start(
                out=xres_sb[:, :, :],
                in_=xres_d.ap().rearrange("(t p) d -> p t d", p=128),
            )
            nc.sync.dma_start(
                out=gb_sb,
                in_=bass.AP(tensor=gamma_d, offset=0, ap=[[0, 128], [1, D]]),
            )
            nc.sync.dma_start(
                out=bb_sb,
                in_=bass.AP(tensor=beta_d, offset=0, ap=[[0, 128], [1, D]]),
            )
            eps_sb = singles.tile([128, 1], F32)
            nc.vector.memset(eps_sb, eps)

            # ---------- projections ----------
            qt_sb = singles.tile([128, 2, L], BF16)
            kt_sb = singles.tile([128, 2, L], BF16)
            v_sb = singles.tile([128, LT, HC * 80], F8)
            nc.vector.memset(
                v_sb.rearrange("p t (h a) -> p t h a", h=HC)[:, :, :, 64:65],
                1.0 / 64.0,
            )

            def proj_one(w_sb, o_sb, m, lc, which, on_act=True):
                ps = psum.tile([128, 512], F32, tag="po", bufs=4,
                               name=f"pj{which}_{m}_{lc}")
                for kd in range(KD // 2):
                    nc.tensor.matmul(
                        ps,
                        lhsT=w_sb[:, 2 * kd : 2 * kd + 2, m * 128 : (m + 1) * 128],
                        rhs=xT_sb[:, 2 * kd : 2 * kd + 2, lc * 512 : (lc + 1) * 512],
                        start=(kd == 0),
                        stop=(kd == KD // 2 - 1),
                        perf_mode=mybir.MatmulPerfMode.DoubleRow,
                    )
                # chains that ride inside the attention window must not put
                # their PSUM->SBUF copy on the exp-saturated ScalarE
                (nc.scalar.copy if on_act else nc.vector.tensor_copy)(
                    out=o_sb[:, m, lc * 512 : (lc + 1) * 512], in_=ps
                )

            # K projection first (scores need all k-positions), then Q;
            # all chains run up front (the attention window is PE-bound,
            # the front idles on DMA waits)
            for lc in range(QCN):
                proj_one(wk_sb, kt_sb, 0, lc, "k")
            proj_one(wq_sb, qt_sb, 0, 0, "q")
            for lc in range(QCN):
                proj_one(wk_sb, kt_sb, 1, lc, "k")
            for lc in range(1, QCN):
                proj_one(wq_sb, qt_sb, 0, lc, "q")
            for lc in range(QCN):
                proj_one(wq_sb, qt_sb, 1, lc, "q")
            for lt in range(LT):
                ps = psum.tile([128, PD], F32, tag="po", bufs=4)
                for kd in range(KD // 2):
                    nc.tensor.matmul(
                        ps,
                        lhsT=xT_sb[:, 2 * kd : 2 * kd + 2, lt * 128 : (lt + 1) * 128],
                        rhs=wv_sb[:, 2 * kd : 2 * kd + 2, :],
                        start=(kd == 0),
                        stop=(kd == KD // 2 - 1),
                        perf_mode=mybir.MatmulPerfMode.DoubleRow,
                    )
                # psum holds 256*v (x and Wv are shipped x16 each)
                nc.scalar.mul(
                    out=v_sb[:, lt, :].rearrange("p (h a) -> p h a", h=HC)[:, :, 0:64],
                    in_=ps.rearrange("p (h a) -> p h a", h=HC),
                    mul=1.0 / 256.0,
                )

            # ---------- attention, per-head-pair AllToAll ----------
            # oT accumulated UNNORMALIZED; per-hp: reciprocal'd softmax
            # denominators ship inside the A2A payload (rows 128-129 of each
            # 130-row shard); the receiver normalizes og before out-proj.
            oT_sb = singles.tile([128, 2, L], F8)
            GS = 2  # score blocks (ktile, h2) per PSUM group of [128, GS*512]
            cc_in = [dram.tile([8, 130, QS], F8, name=f"cc_in{h}") for h in range(2)]
            cc_out = [dram.tile([8 * 130, QS], F8, name=f"cc_out{h}") for h in range(2)]
            og_sb = [singles.tile([128, 4, QS], F8, name=f"og{h}") for h in range(2)]
            S_og = [singles.tile([128, 4, QS], F8, name=f"sog{h}") for h in range(2)]

            def attn_unit(hp, qc):
                po = [
                    psum.tile([65, 512], F32, tag="po", bufs=4,
                              name=f"po_{hp}_{qc}_{h2}")
                    for h2 in range(2)
                ]
                ex_u = work.tile([128, 2, LT, 512], F8, tag="exu", bufs=2,
                                 name=f"exu_{hp}_{qc}")

                def emit_po_pair(t):
                    for h2 in range(2):
                        h_loc = hp * 2 + h2
                        nc.tensor.matmul(
                            po[h2],
                            lhsT=v_sb[:, 2 * t : 2 * t + 2,
                                      h_loc * 80 : h_loc * 80 + 65],
                            rhs=ex_u[:, h2, 2 * t : 2 * t + 2, :],
                            start=(t == 0),
                            stop=(t == LT // 2 - 1),
                            perf_mode=mybir.MatmulPerfMode.DoubleRow,
                        )

                SCK = 10  # kt groups exp'd on ScalarE; the rest on the DVE
                for kt in range(LT):
                    sc = psum.tile(
                        [128, 1024], F32, tag="sc", bufs=2,
                        name=f"sc_{hp}_{qc}_{kt}",
                    )
                    for h2 in range(2):
                        nc.tensor.matmul(
                            sc[:, h2 * 512 : (h2 + 1) * 512],
                            lhsT=kt_sb[
                                64 * h2 : 64 * h2 + 64,
                                hp,
                                kt * 128 : (kt + 1) * 128,
                            ],
                            rhs=qt_sb[
                                64 * h2 : 64 * h2 + 64,
                                hp,
                                qc * 512 : (qc + 1) * 512,
                            ],
                            start=True,
                            stop=True,
                        )
                    scv = sc.rearrange("p (a b) -> p a b", a=2)
                    exv = ex_u[:, :, kt, :]
                    if kt < SCK:
                        nc.scalar.activation(
                            out=exv, in_=scv, func=AF.Exp, scale=0.125 / 16777216.0
                        )
                    else:
                        nc.vector.tensor_scalar(
                            out=exv.bitcast(I8), in0=scv,
                            scalar1=SCHRA_A, scalar2=SCHRA_B,
                            op0=mybir.AluOpType.mult, op1=mybir.AluOpType.add,
                        )
                    if kt >= 3 and kt % 2 == 1:
                        emit_po_pair((kt - 3) // 2)
                emit_po_pair(LT // 2 - 1)

                # denominators: den/64 in bf16, bit-magic reciprocal
                # (bits(1/x) ~= 32503 - bits(x), |rel|<2%), shipped as fp8
                dstage_raw = small.tile([33, 512], BF16, tag="dstage_raw",
                                        bufs=2, name=f"dsr_{hp}_{qc}")
                dstage_bf = small.tile([33, 512], F8, tag="dstage_bf", bufs=2,
                                       name=f"dsb_{hp}_{qc}")
                for h2 in range(2):
                    # clamp: f32->e4m3 converts |x|>240 to +-Inf on device
                    nc.vector.tensor_scalar(
                        out=oT_sb[
                            64 * h2 : 64 * h2 + 64, hp, qc * 512 : (qc + 1) * 512
                        ],
                        in0=po[h2][0:64, :],
                        scalar1=224.0, scalar2=-224.0,
                        op0=mybir.AluOpType.min, op1=mybir.AluOpType.max,
                    )
                    nc.vector.tensor_copy(
                        out=dstage_raw[32 * h2 : 32 * h2 + 1, :],
                        in_=po[h2][64:65, :],
                    )
                Di = dstage_raw.bitcast(mybir.dt.uint16)
                nc.vector.tensor_scalar(
                    out=Di, in0=Di,
                    scalar1=-1.0, scalar2=32503.0,
                    op0=mybir.AluOpType.mult, op1=mybir.AluOpType.add,
                )
                nc.vector.tensor_copy(out=dstage_bf, in_=dstage_raw)
                for p in (qc, qc + 4):
                    nc.sync.dma_start(
                        out=cc_in[hp][p, 0:128, :],
                        in_=oT_sb[
                            :, hp, (p % 4) * QS : (p % 4 + 1) * QS
                        ],
                    )
                    for h2 in range(2):
                        nc.sync.dma_start(
                            out=cc_in[hp][p, 128 + h2, :],
                            in_=dstage_bf[32 * h2 : 32 * h2 + 1, :],
                        )

            def den_chain_and_a2a(hp):
                nc.gpsimd.collective_compute(
                    "AllToAll",
                    mybir.AluOpType.bypass,
                    replica_groups=ALL_RANKS,
                    ins=[cc_in[hp].opt()],
                    outs=[cc_out[hp].opt()],
                )

            def recv_og_dmas(hp, engine):
                # gather the 4 same-batch peers' oT rows + build the broadcast
                # normalizer S from the reciprocal-denominator rows; the
                # same-batch block base is a runtime cc_rank offset
                rank = engine.cc_rank(ALL_RANKS)
                base = (rank // 4) * (4 * 130 * QS)
                engine.dma_start(
                    out=og_sb[hp][:, :, :],
                    in_=bass.AP(
                        tensor=cc_out[hp].tensor,
                        offset=base,
                        ap=[[QS, 128], [130 * QS, 4], [1, QS]],
                    ),
                )
                for h2 in range(2):
                    engine.dma_start(
                        out=S_og[hp][64 * h2 : 64 * h2 + 64, :, :],
                        in_=bass.AP(
                            tensor=cc_out[hp].tensor,
                            offset=base + (128 + h2) * QS,
                            ap=[[0, 64], [130 * QS, 4], [1, QS]],
                        ),
                    )

            def recip_S(hp, eng=None):
                # bf16 magic reciprocal: bits(1/x) ~= K - bits(x); den is
                # narrowly banded (~2100-2600) so K=32503 keeps |rel|<2%,
                # and den only scales the (small) attention contribution
                Si = S_og[hp].bitcast(mybir.dt.uint16)
                (eng or nc.vector).tensor_scalar(
                    out=Si, in0=Si,
                    scalar1=-1.0, scalar2=32503.0,
                    op0=mybir.AluOpType.mult, op1=mybir.AluOpType.add,
                )

            def og_mult(hp, qt=None, eng=None):
                sl = slice(None) if qt is None else slice(qt * 128, (qt + 1) * 128)
                (eng or nc.vector).tensor_mul(
                    out=og_sb[hp][:, :, sl], in0=og_sb[hp][:, :, sl],
                    in1=S_og[hp][:, :, sl],
                )

            # out accumulators in SBUF so the two halves can straddle A2A#2
            out_acc = [
                work.tile([128, D], F32, tag="out_acc", bufs=QT, name=f"oac{qt}")
                for qt in range(QT)
            ]

            def outproj_half(hp):
                for qt in range(QT):
                    outproj_half_qt(hp, qt)

            def outproj_half_qt(hp, qt):
                    for dmt in range(D // 512):
                        ps = psum.tile([128, 512], F32, tag="po", bufs=4,
                                       name=f"op_{hp}_{qt}_{dmt}")
                        for p in range(4):
                            nc.tensor.matmul(
                                ps,
                                lhsT=og_sb[hp][:, p, qt * 128 : (qt + 1) * 128],
                                rhs=wo_sb[:, p * 2 + hp, dmt * 512 : (dmt + 1) * 512],
                                start=(p == 0),
                                stop=(p == 3),
                            )
                        dslice = slice(dmt * 512, (dmt + 1) * 512)
                        # psum = 512*(o . Wo): og ships 64*o, wo ships 8*Wo
                        nc.vector.scalar_tensor_tensor(
                            out=out_acc[qt][:, dslice],
                            in0=ps,
                            scalar=1.0 / 512.0,
                            in1=xres_sb[:, qt, dslice] if hp == 0
                            else out_acc[qt][:, dslice],
                            op0=mybir.AluOpType.mult,
                            op1=mybir.AluOpType.add,
                        )

            for qc in range(QCN):
                attn_unit(0, qc)
            den_chain_and_a2a(0)
            # hp0 receive chain rides the otherwise-idle gpsimd queue between
            # the two A2A triggers: nothing in hp1 attention depends on it,
            # and the A2A#2 trigger is gated by staging semaphores anyway.
            recv_og_dmas(0, nc.gpsimd)
            og_mult(0, eng=nc.gpsimd)
            for qc in range(QCN):
                attn_unit(1, qc)
            den_chain_and_a2a(1)    # gpsimd queue: fires as soon as cc1 staged
            with tc.tile_wait_until(1.0):
                outproj_half(0)     # PE fills the A2A#2 wait
            with tc.tile_wait_until(1.1):
                recv_og_dmas(1, nc.sync)

            # ---------- out projection + residual + layernorm ----------
            BN_STATS_DIM = nc.vector.BN_STATS_DIM
            BN_AGGR_DIM = nc.vector.BN_AGGR_DIM
            tc.tile_set_cur_wait(1.2)
            for qt in range(QT):
                og_mult(1, qt if QT > 1 else None)
                outproj_half_qt(1, qt)
                out_sb = out_acc[qt]
                stats = small.tile([128, SUB, BN_STATS_DIM], F32, tag="stats")
                for s in range(SUB):
                    nc.vector.bn_stats(
                        out=stats[:, s, :],
                        in_=out_sb[:, s * 512 : (s + 1) * 512]
                        if SUB > 1
                        else out_sb[:, :],
                    )
                mv = small.tile([128, BN_AGGR_DIM], F32, tag="mv")
                nc.vector.bn_aggr(out=mv, in_=stats)
                std = small.tile([128, 1], F32, tag="std")
                nc.scalar.activation(
                    out=std, in_=mv[:, 1:2], func=AF.Sqrt, bias=eps_sb, scale=1.0
                )
                rstd = small.tile([128, 1], F32, tag="rstd")
                nc.vector.reciprocal(out=rstd, in_=std)
                out_bf = work.tile([128, D], BF16, tag="out_bf", bufs=2,
                                   name=f"obf{qt}")
                # the final op in the chain writes bf16 so the output DMA
                # ships half the bytes
                n_post = (0 if trivial_gamma else 1) + (0 if trivial_beta else 1)
                nc.vector.tensor_scalar(
                    out=out_bf if n_post == 0 else out_sb,
                    in0=out_sb,
                    scalar1=mv[:, 0:1],
                    scalar2=rstd,
                    op0=mybir.AluOpType.subtract,
                    op1=mybir.AluOpType.mult,
                )
                if not trivial_gamma:
                    nc.vector.tensor_mul(
                        out=out_bf if trivial_beta else out_sb,
                        in0=out_sb, in1=gb_sb,
                    )
                if not trivial_beta:
                    nc.vector.tensor_add(out=out_bf, in0=out_sb, in1=bb_sb)
                nc.sync.dma_start(
                    out=out_d.ap().rearrange("(t p) d -> p t d", p=128)[:, qt, :],
                    in_=out_bf,
                )
    nc.compile()
    return nc


def make_in_maps(x, Wq, Wk, Wv, Wo, ln_gamma, ln_beta, L, D):
    """Host-side sharding: build the 8 per-core input maps."""
    bf = ml_dtypes.bfloat16
    H = 16
    DKV = 64
    PD = 4 * DKV
    QS = L // 4
    # wo blocks: jb = j*2 + hp -> Wo rows for (head-group j, head-pair hp).
    # Uniform across cores: og block j always holds head-group j's oT.
    wo = np.zeros((8 * 128, D), np.float32)
    for j in range(4):
        for hp in range(2):
            r0 = (j * 4 + hp * 2) * DKV
            wo[(j * 2 + hp) * 128 : (j * 2 + hp + 1) * 128] = Wo[r0 : r0 + 128]

    f8 = ml_dtypes.float8_e4m3fn

    def to_f8(a, scale):
        return np.clip(a * scale, -224.0, 224.0).astype(f8)

    wo = to_f8(wo, 8.0)
    in_maps = []
    for c in range(8):
        b, g = c // 4, c % 4
        xT = to_f8(np.ascontiguousarray(x[b].T), 16.0)
        xres = np.ascontiguousarray(x[b, g * QS : (g + 1) * QS]).astype(np.float32)
        cols = slice(g * PD, (g + 1) * PD)
        in_maps.append(
            {
                "xT": xT,
                "xres": xres,
                "wq": to_f8(np.ascontiguousarray(Wq[:, cols]), 256.0),
                "wk": to_f8(np.ascontiguousarray(Wk[:, cols]), 256.0),
                "wv": to_f8(np.ascontiguousarray(Wv[:, cols]), 16.0),
                "wo": wo,
                "gamma": np.ascontiguousarray(ln_gamma[None, :]).astype(np.float32),
                "beta": np.ascontiguousarray(ln_beta[None, :]).astype(np.float32),
            }
        )
    return in_maps


def assemble(results, L, D):
    QS = L // 4
    out = np.zeros((2, L, D), np.float32)
    for c in range(8):
        b, g = c // 4, c % 4
        out[b, g * QS : (g + 1) * QS] = results[c]["out"].astype(np.float32)
    return out


def run(x, Wq, Wk, Wv, Wo, ln_gamma, ln_beta, trace=False):
    B, L, D = x.shape
    nc = build_nc(
        L=L, D=D,
        trivial_gamma=bool(np.all(ln_gamma == 1.0)),
        trivial_beta=bool(np.all(ln_beta == 0.0)),
    )
    in_maps = make_in_maps(x, Wq, Wk, Wv, Wo, ln_gamma, ln_beta, L, D)
    res = bass_utils.run_bass_kernel_spmd(
        nc, in_maps, core_ids=list(range(8)), trace=trace
    )
    return assemble(res.results, L, D), res

def kernel(x, Wq, Wk, Wv, Wo, ln_gamma, ln_beta):
    out, _ = run(
        np.asarray(x, np.float32),
        np.asarray(Wq, np.float32),
        np.asarray(Wk, np.float32),
        np.asarray(Wv, np.float32),
        np.asarray(Wo, np.float32),
        np.asarray(ln_gamma, np.float32),
        np.asarray(ln_beta, np.float32),
    )
    return out


# revision 25
# speedup vs baseline: 1.2614x; 1.2614x over previous
"""MultiHeadSelfAttention Trainium2 kernel (8-core SPMD).

Sharding: batch B=2 x head-groups 4 (4 heads each) -> 8 cores.
Core c: batch b=c//4, head group g=c%4 (heads 4g..4g+4).

Per-core device program:
  1. fp8(e4m3) DoubleRow projections from pre-transposed xT [D, L]
     (host ships x*16, Wq/Wk*256, Wv*16; kd-pairs contract 256 rows/pass),
     all emitted up front where the PE idles on input DMAs:
       QT, KT: [dk=256, L] bf16 (dk on partitions)
       V:      [L, 4*80] fp8, 1/256-column per head -> scaled denominator
  2. Attention per (head-pair hp, q-chunk qc of 512):
       scoresT [128 kpos, 1024] = KT_h.T @ QT_h  (K=64 row-tiled pairs run
       concurrently on the PE array)
       exp -> e4m3, split across engines: ScalarE native Exp (10/16 ktiles)
       + VectorE Schraudolph bit-trick exp (6/16): int8 = s*A + B bitcast
       po_h [65, 512] += V.T @ expT via fp8 DoubleRow over ktile pairs
       (row 64 = den/256); denominators reciprocal'd on the send side with
       a 2-op uint16 magic trick (bits(1/x) ~= 32503 - bits(x), |rel|<2%)
  3. AllToAll (8 ranks, all-fp8 payload): shard p = q-slice (p%4) of po/4 +
     reciprocal rows 128/129 -> receiver keeps only its 4 same-batch peer
     blocks (runtime cc_rank offset into cc_out) and normalizes og = 64*o.
  4. Out-projection (fp8 og x fp8 8*Wo) over the 4 same-batch blocks only,
     fused /512 + residual add, LayerNorm, bf16 output DMA.
"""

import sys

sys.path.insert(0, "/opt/trn_rl_repo")

import numpy as np
import ml_dtypes

import concourse.bass as bass
import concourse.bacc as bacc
import concourse.tile as tile
from concourse import mybir
from concourse import bass_utils
import bass_rust

BF16 = mybir.dt.bfloat16
F32 = mybir.dt.float32
F8 = mybir.dt.float8e4
I8 = mybir.dt.int8
# Schraudolph exp->e4m3-bits on the DVE: bits = round(s_raw*A + B)
# (A = 8*log2(e) * 0.125 score scale; B tuned for zero mean error)
SCHRA_A = 1.4426950408889634 / 16777216.0
SCHRA_B = 55.55
AF = mybir.ActivationFunctionType

_PATCHED = False

ALL_RANKS = [[0, 1, 2, 3, 4, 5, 6, 7]]


def _patch_tile_drain():
    """The installed walrus rejects >1 sem wait on a Drain instruction; split
    the TileContext tail-drain waits across multiple drains."""
    global _PATCHED
    if _PATCHED:
        return
    _PATCHED = True

    def _patched(self, tick_clock, wait_clock):
        from concourse.vector_clock import ScopedClock

        probe = self.nc.sync.drain()
        wait_clock.add_sem_waits(
            probe.ins, ScopedClock({None: tick_clock.global_clock})
        )
        si = probe.ins.sync_info
        waits = list(si.on_wait or []) if si is not None else []
        if len(waits) > 1:
            si.on_wait = [waits[0]]
            for w in waits[1:]:
                d2 = self.nc.sync.drain()
                si2 = d2.ins.sync_info
                if si2 is None:
                    d2.ins.sync_info = bass_rust.SyncInfo(on_wait=[w], on_update=[])
                else:
                    si2.on_wait = [w]
        self.nc.all_engine_barrier()
        assert self.sems is not None
        popped = self.nc._tile_sem_poison_stack.pop()
        assert popped is self._sem_poison
        self.nc.clear_and_free_semaphores(list(self.sems.allocated().values()))
        self.nc.all_engine_barrier()

    tile.TileContext._drain_and_barrier = _patched


def build_nc(L=2048, D=1024, eps=1e-6, trivial_gamma=False, trivial_beta=False):
    """Build the SPMD per-core program. Heads per core = 4, DK=DV=64."""
    _patch_tile_drain()
    HC = 4            # heads per core
    DKV = 64
    PD = HC * DKV     # 256: projected dims per core
    KD = D // 128     # contraction tiles over D
    LT = L // 128     # k-position tiles
    QCN = L // 512    # q chunks of 512
    QS = L // 4       # per-core output rows (q-slice)
    QT = QS // 128    # out row tiles
    HB = 8            # wo row blocks: 4 same-batch peers x 2 head-pairs
    SUB = max(1, D // 512)    # bn_stats subgroups

    nc = bacc.Bacc(num_devices=8, debug=False)

    xT_d = nc.dram_tensor("xT", [D, L], F8, kind="ExternalInput")
    xres_d = nc.dram_tensor("xres", [QS, D], F32, kind="ExternalInput")
    wq_d = nc.dram_tensor("wq", [D, PD], F8, kind="ExternalInput")
    wk_d = nc.dram_tensor("wk", [D, PD], F8, kind="ExternalInput")
    wv_d = nc.dram_tensor("wv", [D, PD], F8, kind="ExternalInput")
    wo_d = nc.dram_tensor("wo", [HB * 128, D], F8, kind="ExternalInput")
    gamma_d = nc.dram_tensor("gamma", [1, D], F32, kind="ExternalInput")
    beta_d = nc.dram_tensor("beta", [1, D], F32, kind="ExternalInput")
    out_d = nc.dram_tensor("out", [QS, D], BF16, kind="ExternalOutput")

    with tile.TileContext(nc) as tc:
        with (
            tc.tile_pool(name="singles", bufs=1) as singles,
            tc.tile_pool(name="work", bufs=3) as work,
            tc.tile_pool(name="small", bufs=4) as small,
            tc.tile_pool(name="psum", bufs=1, space="PSUM") as psum,
            tc.tile_pool(name="dram", bufs=1, space="DRAM") as dram,
        ):
            # ---------- load inputs ----------
            xT_sb = singles.tile([128, KD, L], F8)
            wq_sb = singles.tile([128, KD, PD], F8)
            wk_sb = singles.tile([128, KD, PD], F8)
            wv_sb = singles.tile([128, KD, PD], F8)
            for eng, w_sb, w_d in (
                (nc.scalar, wk_sb, wk_d),
                (nc.scalar, wq_sb, wq_d),
                (nc.gpsimd, wv_sb, wv_d),
            ):
                eng.dma_start(
                    out=w_sb[:, :, :], in_=w_d.ap().rearrange("(t p) m -> p t m", p=128)
                )
            xT_r = xT_d.ap().rearrange("(t p) l -> p t l", p=128)
            for t in range(KD):
                nc.sync.dma_start(out=xT_sb[:, t, :], in_=xT_r[:, t, :])
            wo_sb = singles.tile([128, HB, D], F8)
            xres_sb = singles.tile([128, QT, D], F32)
            gb_sb = singles.tile([128, D], F32)
            bb_sb = singles.tile([128, D], F32)
            # emitted after xT on the same queue: ring FIFO keeps these from
            # stealing HBM bandwidth ahead of the xT critical path
            wo_r = wo_d.ap().rearrange("(t p) n -> p t n", p=128)
            for t in range(HB):
                nc.sync.dma_start(out=wo_sb[:, t, :], in_=wo_r[:, t, :])
            nc.sync.dma_start(
                out=xres_sb[:, :, :],
                in_=xres_d.ap().rearrange("(t p) d -> p t d", p=128),
            )
            nc.sync.dma_start(
                out=gb_sb,
                in_=bass.AP(tensor=gamma_d, offset=0, ap=[[0, 128], [1, D]]),
            )
            nc.sync.dma_start(
                out=bb_sb,
                in_=bass.AP(tensor=beta_d, offset=0, ap=[[0, 128], [1, D]]),
            )
            eps_sb = singles.tile([128, 1], F32)
            nc.vector.memset(eps_sb, eps)

            # ---------- projections ----------
            qt_sb = singles.tile([128, 2, L], BF16)
            kt_sb = singles.tile([128, 2, L], BF16)
            v_sb = singles.tile([128, LT, HC * 80], F8)
            nc.vector.memset(
                v_sb.rearrange("p t (h a) -> p t h a", h=HC)[:, :, :, 64:65],
                1.0 / 256.0,
            )

            def proj_one(w_sb, o_sb, m, lc, which, on_act=True):
                ps = psum.tile([128, 512], F32, tag="po", bufs=4,
                               name=f"pj{which}_{m}_{lc}")
                for kd in range(KD // 2):
                    nc.tensor.matmul(
                        ps,
                        lhsT=w_sb[:, 2 * kd : 2 * kd + 2, m * 128 : (m + 1) * 128],
                        rhs=xT_sb[:, 2 * kd : 2 * kd + 2, lc * 512 : (lc + 1) * 512],
                        start=(kd == 0),
                        stop=(kd == KD // 2 - 1),
                        perf_mode=mybir.MatmulPerfMode.DoubleRow,
                    )
                # chains that ride inside the attention window must not put
                # their PSUM->SBUF copy on the exp-saturated ScalarE
                (nc.scalar.copy if on_act else nc.vector.tensor_copy)(
                    out=o_sb[:, m, lc * 512 : (lc + 1) * 512], in_=ps
                )

            # K projection first (scores need all k-positions), then Q;
            # all chains run up front (the attention window is PE-bound,
            # the front idles on DMA waits)
            for lc in range(QCN):
                proj_one(wk_sb, kt_sb, 0, lc, "k")
            proj_one(wq_sb, qt_sb, 0, 0, "q")
            for lc in range(QCN):
                proj_one(wk_sb, kt_sb, 1, lc, "k")
            for lc in range(1, QCN):
                proj_one(wq_sb, qt_sb, 0, lc, "q")
            for lc in range(QCN):
                proj_one(wq_sb, qt_sb, 1, lc, "q")
            for lt in range(LT):
                ps = psum.tile([128, PD], F32, tag="po", bufs=4)
                for kd in range(KD // 2):
                    nc.tensor.matmul(
                        ps,
                        lhsT=xT_sb[:, 2 * kd : 2 * kd + 2, lt * 128 : (lt + 1) * 128],
                        rhs=wv_sb[:, 2 * kd : 2 * kd + 2, :],
                        start=(kd == 0),
                        stop=(kd == KD // 2 - 1),
                        perf_mode=mybir.MatmulPerfMode.DoubleRow,
                    )
                # psum holds 256*v (x and Wv are shipped x16 each)
                nc.scalar.mul(
                    out=v_sb[:, lt, :].rearrange("p (h a) -> p h a", h=HC)[:, :, 0:64],
                    in_=ps.rearrange("p (h a) -> p h a", h=HC),
                    mul=1.0 / 256.0,
                )

            # ---------- attention, per-head-pair AllToAll ----------
            # oT accumulated UNNORMALIZED; per-hp: reciprocal'd softmax
            # denominators ship inside the A2A payload (rows 128-129 of each
            # 130-row shard); the receiver normalizes og before out-proj.
            oT_sb = singles.tile([128, 2, L], F8)
            GS = 2  # score blocks (ktile, h2) per PSUM group of [128, GS*512]
            cc_in = [dram.tile([8, 130, QS], F8, name=f"cc_in{h}") for h in range(2)]
            cc_out = [dram.tile([8 * 130, QS], F8, name=f"cc_out{h}") for h in range(2)]
            og_sb = [singles.tile([128, 4, QS], F8, name=f"og{h}") for h in range(2)]
            S_og = [singles.tile([128, 4, QS], F8, name=f"sog{h}") for h in range(2)]

            def attn_unit(hp, qc):
                po = [
                    psum.tile([65, 512], F32, tag="po", bufs=4,
                              name=f"po_{hp}_{qc}_{h2}")
                    for h2 in range(2)
                ]
                ex_u = work.tile([128, 2, LT, 512], F8, tag="exu", bufs=2,
                                 name=f"exu_{hp}_{qc}")

                def emit_po_pair(t):
                    for h2 in range(2):
                        h_loc = hp * 2 + h2
                        nc.tensor.matmul(
                            po[h2],
                            lhsT=v_sb[:, 2 * t : 2 * t + 2,
                                      h_loc * 80 : h_loc * 80 + 65],
                            rhs=ex_u[:, h2, 2 * t : 2 * t + 2, :],
                            start=(t == 0),
                            stop=(t == LT // 2 - 1),
                            perf_mode=mybir.MatmulPerfMode.DoubleRow,
                        )

                SCK = 10  # kt groups exp'd on ScalarE; the rest on the DVE
                for kt in range(LT):
                    sc = psum.tile(
                        [128, 1024], F32, tag="sc", bufs=2,
                        name=f"sc_{hp}_{qc}_{kt}",
                    )
                    for h2 in range(2):
                        nc.tensor.matmul(
                            sc[:, h2 * 512 : (h2 + 1) * 512],
                            lhsT=kt_sb[
                                64 * h2 : 64 * h2 + 64,
                                hp,
                                kt * 128 : (kt + 1) * 128,
                            ],
                            rhs=qt_sb[
                                64 * h2 : 64 * h2 + 64,
                                hp,
                                qc * 512 : (qc + 1) * 512,
                            ],
                            start=True,
                            stop=True,
                        )
                    scv = sc.rearrange("p (a b) -> p a b", a=2)
                    exv = ex_u[:, :, kt, :]
                    if kt < SCK:
                        nc.scalar.activation(
                            out=exv, in_=scv, func=AF.Exp, scale=0.125 / 16777216.0
                        )
                    else:
                        nc.vector.tensor_scalar(
                            out=exv.bitcast(I8), in0=scv,
                            scalar1=SCHRA_A, scalar2=SCHRA_B,
                            op0=mybir.AluOpType.mult, op1=mybir.AluOpType.add,
                        )
                    if kt >= 3 and kt % 2 == 1:
                        emit_po_pair((kt - 3) // 2)
                emit_po_pair(LT // 2 - 1)

                # denominators: den/64 in bf16, bit-magic reciprocal
                # (bits(1/x) ~= 32503 - bits(x), |rel|<2%), shipped as fp8
                dstage_raw = small.tile([33, 512], BF16, tag="dstage_raw",
                                        bufs=2, name=f"dsr_{hp}_{qc}")
                dstage_bf = small.tile([33, 512], F8, tag="dstage_bf", bufs=2,
                                       name=f"dsb_{hp}_{qc}")
                for h2 in range(2):
                    # ship po/4 (sigma ~8.5) so f32->e4m3 never saturates;
                    # min-clamp is belt and suspenders for the + tail
                    nc.vector.tensor_scalar(
                        out=oT_sb[
                            64 * h2 : 64 * h2 + 64, hp, qc * 512 : (qc + 1) * 512
                        ],
                        in0=po[h2][0:64, :],
                        scalar1=0.25, scalar2=224.0,
                        op0=mybir.AluOpType.mult, op1=mybir.AluOpType.min,
                    )
                    nc.vector.tensor_copy(
                        out=dstage_raw[32 * h2 : 32 * h2 + 1, :],
                        in_=po[h2][64:65, :],
                    )
                Di = dstage_raw.bitcast(mybir.dt.uint16)
                nc.vector.tensor_scalar(
                    out=Di, in0=Di,
                    scalar1=-1.0, scalar2=32503.0,
                    op0=mybir.AluOpType.mult, op1=mybir.AluOpType.add,
                )
                nc.vector.tensor_copy(out=dstage_bf, in_=dstage_raw)
                for p in (qc, qc + 4):
                    nc.sync.dma_start(
                        out=cc_in[hp][p, 0:128, :],
                        in_=oT_sb[
                            :, hp, (p % 4) * QS : (p % 4 + 1) * QS
                        ],
                    )
                    for h2 in range(2):
                        nc.sync.dma_start(
                            out=cc_in[hp][p, 128 + h2, :],
                            in_=dstage_bf[32 * h2 : 32 * h2 + 1, :],
                        )

            def den_chain_and_a2a(hp):
                nc.gpsimd.collective_compute(
                    "AllToAll",
                    mybir.AluOpType.bypass,
                    replica_groups=ALL_RANKS,
                    ins=[cc_in[hp].opt()],
                    outs=[cc_out[hp].opt()],
                )

            def recv_og_dmas(hp, engine):
                # gather the 4 same-batch peers' oT rows + build the broadcast
                # normalizer S from the reciprocal-denominator rows; the
                # same-batch block base is a runtime cc_rank offset
                rank = engine.cc_rank(ALL_RANKS)
                base = (rank // 4) * (4 * 130 * QS)
                engine.dma_start(
                    out=og_sb[hp][:, :, :],
                    in_=bass.AP(
                        tensor=cc_out[hp].tensor,
                        offset=base,
                        ap=[[QS, 128], [130 * QS, 4], [1, QS]],
                    ),
                )
                for h2 in range(2):
                    engine.dma_start(
                        out=S_og[hp][64 * h2 : 64 * h2 + 64, :, :],
                        in_=bass.AP(
                            tensor=cc_out[hp].tensor,
                            offset=base + (128 + h2) * QS,
                            ap=[[0, 64], [130 * QS, 4], [1, QS]],
                        ),
                    )

            def recip_S(hp, eng=None):
                # bf16 magic reciprocal: bits(1/x) ~= K - bits(x); den is
                # narrowly banded (~2100-2600) so K=32503 keeps |rel|<2%,
                # and den only scales the (small) attention contribution
                Si = S_og[hp].bitcast(mybir.dt.uint16)
                (eng or nc.vector).tensor_scalar(
                    out=Si, in0=Si,
                    scalar1=-1.0, scalar2=32503.0,
                    op0=mybir.AluOpType.mult, op1=mybir.AluOpType.add,
                )

            def og_mult(hp, qt=None, eng=None):
                sl = slice(None) if qt is None else slice(qt * 128, (qt + 1) * 128)
                (eng or nc.vector).tensor_mul(
                    out=og_sb[hp][:, :, sl], in0=og_sb[hp][:, :, sl],
                    in1=S_og[hp][:, :, sl],
                )

            # out accumulators in SBUF so the two halves can straddle A2A#2
            out_acc = [
                work.tile([128, D], F32, tag="out_acc", bufs=QT, name=f"oac{qt}")
                for qt in range(QT)
            ]

            def outproj_half(hp):
                for qt in range(QT):
                    outproj_half_qt(hp, qt)

            def outproj_half_qt(hp, qt):
                    for dmt in range(D // 512):
                        ps = psum.tile([128, 512], F32, tag="po", bufs=4,
                                       name=f"op_{hp}_{qt}_{dmt}")
                        for p in range(4):
                            nc.tensor.matmul(
                                ps,
                                lhsT=og_sb[hp][:, p, qt * 128 : (qt + 1) * 128],
                                rhs=wo_sb[:, p * 2 + hp, dmt * 512 : (dmt + 1) * 512],
                                start=(p == 0),
                                stop=(p == 3),
                            )
                        dslice = slice(dmt * 512, (dmt + 1) * 512)
                        # psum = 512*(o . Wo): og ships 64*o, wo ships 8*Wo
                        nc.vector.scalar_tensor_tensor(
                            out=out_acc[qt][:, dslice],
                            in0=ps,
                            scalar=1.0 / 512.0,
                            in1=xres_sb[:, qt, dslice] if hp == 0
                            else out_acc[qt][:, dslice],
                            op0=mybir.AluOpType.mult,
                            op1=mybir.AluOpType.add,
                        )

            for qc in range(QCN):
                attn_unit(0, qc)
            den_chain_and_a2a(0)
            # hp0 receive chain rides the otherwise-idle gpsimd queue between
            # the two A2A triggers: nothing in hp1 attention depends on it,
            # and the A2A#2 trigger is gated by staging semaphores anyway.
            recv_og_dmas(0, nc.gpsimd)
            og_mult(0, eng=nc.gpsimd)
            for qc in range(QCN):
                attn_unit(1, qc)
            den_chain_and_a2a(1)    # gpsimd queue: fires as soon as cc1 staged
            with tc.tile_wait_until(1.0):
                outproj_half(0)     # PE fills the A2A#2 wait
            with tc.tile_wait_until(1.1):
                recv_og_dmas(1, nc.sync)

            # ---------- out projection + residual + layernorm ----------
            BN_STATS_DIM = nc.vector.BN_STATS_DIM
            BN_AGGR_DIM = nc.vector.BN_AGGR_DIM
            tc.tile_set_cur_wait(1.2)
            for qt in range(QT):
                og_mult(1, qt if QT > 1 else None)
                outproj_half_qt(1, qt)
                out_sb = out_acc[qt]
                stats = small.tile([128, SUB, BN_STATS_DIM], F32, tag="stats")
                for s in range(SUB):
                    nc.vector.bn_stats(
                        out=stats[:, s, :],
                        in_=out_sb[:, s * 512 : (s + 1) * 512]
                        if SUB > 1
                        else out_sb[:, :],
                    )
                mv = small.tile([128, BN_AGGR_DIM], F32, tag="mv")
                nc.vector.bn_aggr(out=mv, in_=stats)
                std = small.tile([128, 1], F32, tag="std")
                nc.scalar.activation(
                    out=std, in_=mv[:, 1:2], func=AF.Sqrt, bias=eps_sb, scale=1.0
                )
                rstd = small.tile([128, 1], F32, tag="rstd")
                nc.vector.reciprocal(out=rstd, in_=std)
                out_bf = work.tile([128, D], BF16, tag="out_bf", bufs=2,
                                   name=f"obf{qt}")
                # the final op in the chain writes bf16 so the output DMA
                # ships half the bytes
                n_post = (0 if trivial_gamma else 1) + (0 if trivial_beta else 1)
                nc.vector.tensor_scalar(
                    out=out_bf if n_post == 0 else out_sb,
                    in0=out_sb,
                    scalar1=mv[:, 0:1],
                    scalar2=rstd,
                    op0=mybir.AluOpType.subtract,
                    op1=mybir.AluOpType.mult,
                )
                if not trivial_gamma:
                    nc.vector.tensor_mul(
                        out=out_bf if trivial_beta else out_sb,
                        in0=out_sb, in1=gb_sb,
                    )
                if not trivial_beta:
                    nc.vector.tensor_add(out=out_bf, in0=out_sb, in1=bb_sb)
                nc.sync.dma_start(
                    out=out_d.ap().rearrange("(t p) d -> p t d", p=128)[:, qt, :],
                    in_=out_bf,
                )
    nc.compile()
    return nc


def make_in_maps(x, Wq, Wk, Wv, Wo, ln_gamma, ln_beta, L, D):
    """Host-side sharding: build the 8 per-core input maps."""
    bf = ml_dtypes.bfloat16
    H = 16
    DKV = 64
    PD = 4 * DKV
    QS = L // 4
    # wo blocks: jb = j*2 + hp -> Wo rows for (head-group j, head-pair hp).
    # Uniform across cores: og block j always holds head-group j's oT.
    wo = np.zeros((8 * 128, D), np.float32)
    for j in range(4):
        for hp in range(2):
            r0 = (j * 4 + hp * 2) * DKV
            wo[(j * 2 + hp) * 128 : (j * 2 + hp + 1) * 128] = Wo[r0 : r0 + 128]

    f8 = ml_dtypes.float8_e4m3fn

    def to_f8(a, scale):
        return np.clip(a * scale, -224.0, 224.0).astype(f8)

    wo = to_f8(wo, 8.0)
    in_maps = []
    for c in range(8):
        b, g = c // 4, c % 4
        xT = to_f8(np.ascontiguousarray(x[b].T), 16.0)
        xres = np.ascontiguousarray(x[b, g * QS : (g + 1) * QS]).astype(np.float32)
        cols = slice(g * PD, (g + 1) * PD)
        in_maps.append(
            {
                "xT": xT,
                "xres": xres,
                "wq": to_f8(np.ascontiguousarray(Wq[:, cols]), 256.0),
                "wk": to_f8(np.ascontiguousarray(Wk[:, cols]), 256.0),
                "wv": to_f8(np.ascontiguousarray(Wv[:, cols]), 16.0),
                "wo": wo,
                "gamma": np.ascontiguousarray(ln_gamma[None, :]).astype(np.float32),
                "beta": np.ascontiguousarray(ln_beta[None, :]).astype(np.float32),
            }
        )
    return in_maps


def assemble(results, L, D):
    QS = L // 4
    out = np.zeros((2, L, D), np.float32)
    for c in range(8):
        b, g = c // 4, c % 4
        out[b, g * QS : (g + 1) * QS] = results[c]["out"].astype(np.float32)
    return out


def run(x, Wq, Wk, Wv, Wo, ln_gamma, ln_beta, trace=False):
    B, L, D = x.shape
    nc = build_nc(
        L=L, D=D,
        trivial_gamma=bool(np.all(ln_gamma == 1.0)),
        trivial_beta=bool(np.all(ln_beta == 0.0)),
    )
    in_maps = make_in_maps(x, Wq, Wk, Wv, Wo, ln_gamma, ln_beta, L, D)
    res = bass_utils.run_bass_kernel_spmd(
        nc, in_maps, core_ids=list(range(8)), trace=trace
    )
    return assemble(res.results, L, D), res

def kernel(x, Wq, Wk, Wv, Wo, ln_gamma, ln_beta):
    out, _ = run(
        np.asarray(x, np.float32),
        np.asarray(Wq, np.float32),
        np.asarray(Wk, np.float32),
        np.asarray(Wv, np.float32),
        np.asarray(Wo, np.float32),
        np.asarray(ln_gamma, np.float32),
        np.asarray(ln_beta, np.float32),
    )
    return out


# revision 26
# speedup vs baseline: 1.3855x; 1.0984x over previous
"""MultiHeadSelfAttention Trainium2 kernel (8-core SPMD).

Sharding: batch B=2 x head-groups 4 (4 heads each) -> 8 cores.
Core c: batch b=c//4, head group g=c%4 (heads 4g..4g+4).

Per-core device program:
  1. fp8(e4m3) DoubleRow projections from pre-transposed xT [D, L]
     (host ships x*16, Wq/Wk*256, Wv*16; kd-pairs contract 256 rows/pass),
     all emitted up front where the PE idles on input DMAs:
       QT, KT: [dk=256, L] bf16 (dk on partitions)
       V:      [L, 4*80] fp8, 1/256-column per head -> scaled denominator
  2. Attention per (head-pair hp, q-chunk qc of 512):
       scoresT [128 kpos, 1024] = KT_h.T @ QT_h  (K=64 row-tiled pairs run
       concurrently on the PE array)
       exp -> e4m3, split across engines: ScalarE native Exp (10/16 ktiles)
       + VectorE Schraudolph bit-trick exp (6/16): int8 = s*A + B bitcast
       po_h [65, 512] += V.T @ expT via fp8 DoubleRow over ktile pairs
       (row 64 = den/256); denominators reciprocal'd on the send side with
       a 2-op uint16 magic trick (bits(1/x) ~= 32503 - bits(x), |rel|<2%)
  3. AllToAll (8 ranks, all-fp8 payload): shard p = q-slice (p%4) of po/4 +
     reciprocal rows 128/129 -> receiver keeps only its 4 same-batch peer
     blocks (runtime cc_rank offset into cc_out) and normalizes og = 64*o.
  4. Out-projection (fp8 og x fp8 8*Wo) over the 4 same-batch blocks only,
     fused /512 + residual add, LayerNorm, bf16 output DMA.
"""

import sys

sys.path.insert(0, "/opt/trn_rl_repo")

import numpy as np
import ml_dtypes

import concourse.bass as bass
import concourse.bacc as bacc
import concourse.tile as tile
from concourse import mybir
from concourse import bass_utils
import bass_rust

BF16 = mybir.dt.bfloat16
F32 = mybir.dt.float32
F8 = mybir.dt.float8e4
I8 = mybir.dt.int8
# Schraudolph exp->e4m3-bits on the DVE: bits = round(s_raw*A + B)
# (A = 8*log2(e) * 0.125 score scale; B tuned for zero mean error)
SCHRA_A = 1.4426950408889634 / 16777216.0
SCHRA_B = 55.55
AF = mybir.ActivationFunctionType

_PATCHED = False

ALL_RANKS = [[0, 1, 2, 3, 4, 5, 6, 7]]


def _patch_tile_drain():
    """The installed walrus rejects >1 sem wait on a Drain instruction; split
    the TileContext tail-drain waits across multiple drains."""
    global _PATCHED
    if _PATCHED:
        return
    _PATCHED = True

    def _patched(self, tick_clock, wait_clock):
        from concourse.vector_clock import ScopedClock

        probe = self.nc.sync.drain()
        wait_clock.add_sem_waits(
            probe.ins, ScopedClock({None: tick_clock.global_clock})
        )
        si = probe.ins.sync_info
        waits = list(si.on_wait or []) if si is not None else []
        if len(waits) > 1:
            si.on_wait = [waits[0]]
            for w in waits[1:]:
                d2 = self.nc.sync.drain()
                si2 = d2.ins.sync_info
                if si2 is None:
                    d2.ins.sync_info = bass_rust.SyncInfo(on_wait=[w], on_update=[])
                else:
                    si2.on_wait = [w]
        self.nc.all_engine_barrier()
        assert self.sems is not None
        popped = self.nc._tile_sem_poison_stack.pop()
        assert popped is self._sem_poison
        self.nc.clear_and_free_semaphores(list(self.sems.allocated().values()))
        self.nc.all_engine_barrier()

    tile.TileContext._drain_and_barrier = _patched


def build_nc(L=2048, D=1024, eps=1e-6, trivial_gamma=False, trivial_beta=False):
    """Build the SPMD per-core program. Heads per core = 4, DK=DV=64."""
    _patch_tile_drain()
    HC = 4            # heads per core
    DKV = 64
    PD = HC * DKV     # 256: projected dims per core
    KD = D // 128     # contraction tiles over D
    LT = L // 128     # k-position tiles
    QCN = L // 512    # q chunks of 512
    QS = L // 4       # per-core output rows (q-slice)
    QT = QS // 128    # out row tiles
    HB = 8            # wo row blocks: 4 same-batch peers x 2 head-pairs
    SUB = max(1, D // 512)    # bn_stats subgroups

    nc = bacc.Bacc(num_devices=8, debug=False)

    xT_d = nc.dram_tensor("xT", [D, L], F8, kind="ExternalInput")
    xres_d = nc.dram_tensor("xres", [QS, D], F32, kind="ExternalInput")
    wq_d = nc.dram_tensor("wq", [D, PD], F8, kind="ExternalInput")
    wk_d = nc.dram_tensor("wk", [D, PD], F8, kind="ExternalInput")
    wv_d = nc.dram_tensor("wv", [D, PD], F8, kind="ExternalInput")
    wo_d = nc.dram_tensor("wo", [HB * 128, D], F8, kind="ExternalInput")
    gamma_d = nc.dram_tensor("gamma", [1, D], F32, kind="ExternalInput")
    beta_d = nc.dram_tensor("beta", [1, D], F32, kind="ExternalInput")
    out_d = nc.dram_tensor("out", [QS, D], BF16, kind="ExternalOutput")

    with tile.TileContext(nc) as tc:
        with (
            tc.tile_pool(name="singles", bufs=1) as singles,
            tc.tile_pool(name="work", bufs=3) as work,
            tc.tile_pool(name="small", bufs=4) as small,
            tc.tile_pool(name="psum", bufs=1, space="PSUM") as psum,
            tc.tile_pool(name="dram", bufs=1, space="DRAM") as dram,
        ):
            # ---------- load inputs ----------
            xT_sb = singles.tile([128, KD, L], F8)
            wq_sb = singles.tile([128, KD, PD], F8)
            wk_sb = singles.tile([128, KD, PD], F8)
            wv_sb = singles.tile([128, KD, PD], F8)
            for eng, w_sb, w_d in (
                (nc.scalar, wk_sb, wk_d),
                (nc.scalar, wq_sb, wq_d),
                (nc.gpsimd, wv_sb, wv_d),
            ):
                eng.dma_start(
                    out=w_sb[:, :, :], in_=w_d.ap().rearrange("(t p) m -> p t m", p=128)
                )
            xT_r = xT_d.ap().rearrange("(t p) l -> p t l", p=128)
            for t in range(KD):
                nc.sync.dma_start(out=xT_sb[:, t, :], in_=xT_r[:, t, :])
            wo_sb = singles.tile([128, HB, D], F8)
            xres_sb = singles.tile([128, QT, D], F32)
            gb_sb = singles.tile([128, D], F32)
            bb_sb = singles.tile([128, D], F32)
            # emitted after xT on the same queue: ring FIFO keeps these from
            # stealing HBM bandwidth ahead of the xT critical path
            wo_r = wo_d.ap().rearrange("(t p) n -> p t n", p=128)
            for t in range(HB):
                nc.sync.dma_start(out=wo_sb[:, t, :], in_=wo_r[:, t, :])
            nc.sync.dma_start(
                out=xres_sb[:, :, :],
                in_=xres_d.ap().rearrange("(t p) d -> p t d", p=128),
            )
            nc.sync.dma_start(
                out=gb_sb,
                in_=bass.AP(tensor=gamma_d, offset=0, ap=[[0, 128], [1, D]]),
            )
            nc.sync.dma_start(
                out=bb_sb,
                in_=bass.AP(tensor=beta_d, offset=0, ap=[[0, 128], [1, D]]),
            )
            eps_sb = singles.tile([128, 1], F32)
            nc.vector.memset(eps_sb, eps)

            # ---------- projections ----------
            qt_sb = singles.tile([128, 2, L], BF16)
            kt_sb = singles.tile([128, 2, L], BF16)
            v_sb = singles.tile([128, LT, HC * 80], F8)
            nc.vector.memset(
                v_sb.rearrange("p t (h a) -> p t h a", h=HC)[:, :, :, 64:65],
                1.0 / 256.0,
            )

            def proj_one(w_sb, o_sb, m, lc, which, on_act=True):
                ps = psum.tile([128, 512], F32, tag="po", bufs=4,
                               name=f"pj{which}_{m}_{lc}")
                for kd in range(KD // 2):
                    nc.tensor.matmul(
                        ps,
                        lhsT=w_sb[:, 2 * kd : 2 * kd + 2, m * 128 : (m + 1) * 128],
                        rhs=xT_sb[:, 2 * kd : 2 * kd + 2, lc * 512 : (lc + 1) * 512],
                        start=(kd == 0),
                        stop=(kd == KD // 2 - 1),
                        perf_mode=mybir.MatmulPerfMode.DoubleRow,
                    )
                # chains that ride inside the attention window must not put
                # their PSUM->SBUF copy on the exp-saturated ScalarE
                (nc.scalar.copy if on_act else nc.vector.tensor_copy)(
                    out=o_sb[:, m, lc * 512 : (lc + 1) * 512], in_=ps
                )

            # K projection first (scores need all k-positions), then Q;
            # all chains run up front (the attention window is PE-bound,
            # the front idles on DMA waits)
            for lc in range(QCN):
                proj_one(wk_sb, kt_sb, 0, lc, "k")
            proj_one(wq_sb, qt_sb, 0, 0, "q")
            for lc in range(QCN):
                proj_one(wk_sb, kt_sb, 1, lc, "k")
            for lc in range(1, QCN):
                proj_one(wq_sb, qt_sb, 0, lc, "q")
            for lc in range(QCN):
                proj_one(wq_sb, qt_sb, 1, lc, "q")
            for lt in range(LT):
                ps = psum.tile([128, PD], F32, tag="po", bufs=4)
                for kd in range(KD // 2):
                    nc.tensor.matmul(
                        ps,
                        lhsT=xT_sb[:, 2 * kd : 2 * kd + 2, lt * 128 : (lt + 1) * 128],
                        rhs=wv_sb[:, 2 * kd : 2 * kd + 2, :],
                        start=(kd == 0),
                        stop=(kd == KD // 2 - 1),
                        perf_mode=mybir.MatmulPerfMode.DoubleRow,
                    )
                # psum holds 256*v (x and Wv are shipped x16 each)
                nc.scalar.mul(
                    out=v_sb[:, lt, :].rearrange("p (h a) -> p h a", h=HC)[:, :, 0:64],
                    in_=ps.rearrange("p (h a) -> p h a", h=HC),
                    mul=1.0 / 256.0,
                )

            # ---------- attention, per-head-pair AllToAll ----------
            # oT accumulated UNNORMALIZED; per-hp: reciprocal'd softmax
            # denominators ship inside the A2A payload (rows 128-129 of each
            # 130-row shard); the receiver normalizes og before out-proj.
            oT_sb = singles.tile([128, 2, L], F8)
            GS = 2  # score blocks (ktile, h2) per PSUM group of [128, GS*512]
            cc_in = [dram.tile([8, 130, QS], F8, name=f"cc_in{h}") for h in range(2)]
            cc_out = [dram.tile([8 * 130, QS], F8, name=f"cc_out{h}") for h in range(2)]
            og_sb = [singles.tile([128, 4, QS], F8, name=f"og{h}") for h in range(2)]
            S_og = [singles.tile([128, 4, QS], F8, name=f"sog{h}") for h in range(2)]

            def attn_unit(hp, qc):
                po = [
                    psum.tile([65, 512], F32, tag="po", bufs=4,
                              name=f"po_{hp}_{qc}_{h2}")
                    for h2 in range(2)
                ]
                ex_u = work.tile([128, 2, LT, 512], F8, tag="exu", bufs=2,
                                 name=f"exu_{hp}_{qc}")

                def emit_po_pair(t):
                    for h2 in range(2):
                        h_loc = hp * 2 + h2
                        nc.tensor.matmul(
                            po[h2],
                            lhsT=v_sb[:, 2 * t : 2 * t + 2,
                                      h_loc * 80 : h_loc * 80 + 65],
                            rhs=ex_u[:, h2, 2 * t : 2 * t + 2, :],
                            start=(t == 0),
                            stop=(t == LT // 2 - 1),
                            perf_mode=mybir.MatmulPerfMode.DoubleRow,
                        )

                # exp engine assignment INTERLEAVED so the 2-deep score
                # buffer (which forces kt-ordered exp) lets ScalarE and DVE
                # ping-pong instead of running as two serial blocks
                DVE_KT = {2, 5, 8, 10, 12, 14}
                for kt in range(LT):
                    sc = psum.tile(
                        [128, 1024], F32, tag="sc", bufs=2,
                        name=f"sc_{hp}_{qc}_{kt}",
                    )
                    for h2 in range(2):
                        nc.tensor.matmul(
                            sc[:, h2 * 512 : (h2 + 1) * 512],
                            lhsT=kt_sb[
                                64 * h2 : 64 * h2 + 64,
                                hp,
                                kt * 128 : (kt + 1) * 128,
                            ],
                            rhs=qt_sb[
                                64 * h2 : 64 * h2 + 64,
                                hp,
                                qc * 512 : (qc + 1) * 512,
                            ],
                            start=True,
                            stop=True,
                        )
                    scv = sc.rearrange("p (a b) -> p a b", a=2)
                    exv = ex_u[:, :, kt, :]
                    if kt not in DVE_KT:
                        nc.scalar.activation(
                            out=exv, in_=scv, func=AF.Exp, scale=0.125 / 16777216.0
                        )
                    else:
                        nc.vector.tensor_scalar(
                            out=exv.bitcast(I8), in0=scv,
                            scalar1=SCHRA_A, scalar2=SCHRA_B,
                            op0=mybir.AluOpType.mult, op1=mybir.AluOpType.add,
                        )
                    if kt >= 3 and kt % 2 == 1:
                        emit_po_pair((kt - 3) // 2)
                emit_po_pair(LT // 2 - 1)

                # denominators: den/64 in bf16, bit-magic reciprocal
                # (bits(1/x) ~= 32503 - bits(x), |rel|<2%), shipped as fp8
                dstage_raw = small.tile([33, 512], BF16, tag="dstage_raw",
                                        bufs=2, name=f"dsr_{hp}_{qc}")
                dstage_bf = small.tile([33, 512], F8, tag="dstage_bf", bufs=2,
                                       name=f"dsb_{hp}_{qc}")
                for h2 in range(2):
                    # ship po/4 (sigma ~8.5) so f32->e4m3 never saturates;
                    # min-clamp is belt and suspenders for the + tail
                    nc.vector.tensor_scalar(
                        out=oT_sb[
                            64 * h2 : 64 * h2 + 64, hp, qc * 512 : (qc + 1) * 512
                        ],
                        in0=po[h2][0:64, :],
                        scalar1=0.25, scalar2=224.0,
                        op0=mybir.AluOpType.mult, op1=mybir.AluOpType.min,
                    )
                    nc.vector.tensor_copy(
                        out=dstage_raw[32 * h2 : 32 * h2 + 1, :],
                        in_=po[h2][64:65, :],
                    )
                Di = dstage_raw.bitcast(mybir.dt.uint16)
                nc.vector.tensor_scalar(
                    out=Di, in0=Di,
                    scalar1=-1.0, scalar2=32503.0,
                    op0=mybir.AluOpType.mult, op1=mybir.AluOpType.add,
                )
                nc.vector.tensor_copy(out=dstage_bf, in_=dstage_raw)
                for p in (qc, qc + 4):
                    nc.sync.dma_start(
                        out=cc_in[hp][p, 0:128, :],
                        in_=oT_sb[
                            :, hp, (p % 4) * QS : (p % 4 + 1) * QS
                        ],
                    )
                    for h2 in range(2):
                        nc.sync.dma_start(
                            out=cc_in[hp][p, 128 + h2, :],
                            in_=dstage_bf[32 * h2 : 32 * h2 + 1, :],
                        )

            def den_chain_and_a2a(hp):
                nc.gpsimd.collective_compute(
                    "AllToAll",
                    mybir.AluOpType.bypass,
                    replica_groups=ALL_RANKS,
                    ins=[cc_in[hp].opt()],
                    outs=[cc_out[hp].opt()],
                )

            def recv_og_dmas(hp, engine):
                # gather the 4 same-batch peers' oT rows + build the broadcast
                # normalizer S from the reciprocal-denominator rows; the
                # same-batch block base is a runtime cc_rank offset
                rank = engine.cc_rank(ALL_RANKS)
                base = (rank // 4) * (4 * 130 * QS)
                engine.dma_start(
                    out=og_sb[hp][:, :, :],
                    in_=bass.AP(
                        tensor=cc_out[hp].tensor,
                        offset=base,
                        ap=[[QS, 128], [130 * QS, 4], [1, QS]],
                    ),
                )
                for h2 in range(2):
                    engine.dma_start(
                        out=S_og[hp][64 * h2 : 64 * h2 + 64, :, :],
                        in_=bass.AP(
                            tensor=cc_out[hp].tensor,
                            offset=base + (128 + h2) * QS,
                            ap=[[0, 64], [130 * QS, 4], [1, QS]],
                        ),
                    )

            def recip_S(hp, eng=None):
                # bf16 magic reciprocal: bits(1/x) ~= K - bits(x); den is
                # narrowly banded (~2100-2600) so K=32503 keeps |rel|<2%,
                # and den only scales the (small) attention contribution
                Si = S_og[hp].bitcast(mybir.dt.uint16)
                (eng or nc.vector).tensor_scalar(
                    out=Si, in0=Si,
                    scalar1=-1.0, scalar2=32503.0,
                    op0=mybir.AluOpType.mult, op1=mybir.AluOpType.add,
                )

            def og_mult(hp, qt=None, eng=None):
                sl = slice(None) if qt is None else slice(qt * 128, (qt + 1) * 128)
                (eng or nc.vector).tensor_mul(
                    out=og_sb[hp][:, :, sl], in0=og_sb[hp][:, :, sl],
                    in1=S_og[hp][:, :, sl],
                )

            # out accumulators in SBUF so the two halves can straddle A2A#2
            out_acc = [
                work.tile([128, D], F32, tag="out_acc", bufs=QT, name=f"oac{qt}")
                for qt in range(QT)
            ]

            def outproj_half(hp):
                for qt in range(QT):
                    outproj_half_qt(hp, qt)

            def outproj_half_qt(hp, qt):
                    for dmt in range(D // 512):
                        ps = psum.tile([128, 512], F32, tag="po", bufs=4,
                                       name=f"op_{hp}_{qt}_{dmt}")
                        for p in range(4):
                            nc.tensor.matmul(
                                ps,
                                lhsT=og_sb[hp][:, p, qt * 128 : (qt + 1) * 128],
                                rhs=wo_sb[:, p * 2 + hp, dmt * 512 : (dmt + 1) * 512],
                                start=(p == 0),
                                stop=(p == 3),
                            )
                        dslice = slice(dmt * 512, (dmt + 1) * 512)
                        # psum = 512*(o . Wo): og ships 64*o, wo ships 8*Wo
                        nc.vector.scalar_tensor_tensor(
                            out=out_acc[qt][:, dslice],
                            in0=ps,
                            scalar=1.0 / 512.0,
                            in1=xres_sb[:, qt, dslice] if hp == 0
                            else out_acc[qt][:, dslice],
                            op0=mybir.AluOpType.mult,
                            op1=mybir.AluOpType.add,
                        )

            for qc in range(QCN):
                attn_unit(0, qc)
            den_chain_and_a2a(0)
            # hp0 receive chain rides the otherwise-idle gpsimd queue between
            # the two A2A triggers: nothing in hp1 attention depends on it,
            # and the A2A#2 trigger is gated by staging semaphores anyway.
            recv_og_dmas(0, nc.gpsimd)
            og_mult(0, eng=nc.gpsimd)
            for qc in range(QCN):
                attn_unit(1, qc)
            den_chain_and_a2a(1)    # gpsimd queue: fires as soon as cc1 staged
            with tc.tile_wait_until(1.0):
                outproj_half(0)     # PE fills the A2A#2 wait
            with tc.tile_wait_until(1.1):
                recv_og_dmas(1, nc.sync)

            # ---------- out projection + residual + layernorm ----------
            BN_STATS_DIM = nc.vector.BN_STATS_DIM
            BN_AGGR_DIM = nc.vector.BN_AGGR_DIM
            tc.tile_set_cur_wait(1.2)
            for qt in range(QT):
                og_mult(1, qt if QT > 1 else None)
                outproj_half_qt(1, qt)
                out_sb = out_acc[qt]
                stats = small.tile([128, SUB, BN_STATS_DIM], F32, tag="stats")
                for s in range(SUB):
                    nc.vector.bn_stats(
                        out=stats[:, s, :],
                        in_=out_sb[:, s * 512 : (s + 1) * 512]
                        if SUB > 1
                        else out_sb[:, :],
                    )
                mv = small.tile([128, BN_AGGR_DIM], F32, tag="mv")
                nc.vector.bn_aggr(out=mv, in_=stats)
                std = small.tile([128, 1], F32, tag="std")
                nc.scalar.activation(
                    out=std, in_=mv[:, 1:2], func=AF.Sqrt, bias=eps_sb, scale=1.0
                )
                rstd = small.tile([128, 1], F32, tag="rstd")
                nc.vector.reciprocal(out=rstd, in_=std)
                out_bf = work.tile([128, D], BF16, tag="out_bf", bufs=2,
                                   name=f"obf{qt}")
                # the final op in the chain writes bf16 so the output DMA
                # ships half the bytes
                n_post = (0 if trivial_gamma else 1) + (0 if trivial_beta else 1)
                nc.vector.tensor_scalar(
                    out=out_bf if n_post == 0 else out_sb,
                    in0=out_sb,
                    scalar1=mv[:, 0:1],
                    scalar2=rstd,
                    op0=mybir.AluOpType.subtract,
                    op1=mybir.AluOpType.mult,
                )
                if not trivial_gamma:
                    nc.vector.tensor_mul(
                        out=out_bf if trivial_beta else out_sb,
                        in0=out_sb, in1=gb_sb,
                    )
                if not trivial_beta:
                    nc.vector.tensor_add(out=out_bf, in0=out_sb, in1=bb_sb)
                nc.sync.dma_start(
                    out=out_d.ap().rearrange("(t p) d -> p t d", p=128)[:, qt, :],
                    in_=out_bf,
                )
    nc.compile()
    return nc


def make_in_maps(x, Wq, Wk, Wv, Wo, ln_gamma, ln_beta, L, D):
    """Host-side sharding: build the 8 per-core input maps."""
    bf = ml_dtypes.bfloat16
    H = 16
    DKV = 64
    PD = 4 * DKV
    QS = L // 4
    # wo blocks: jb = j*2 + hp -> Wo rows for (head-group j, head-pair hp).
    # Uniform across cores: og block j always holds head-group j's oT.
    wo = np.zeros((8 * 128, D), np.float32)
    for j in range(4):
        for hp in range(2):
            r0 = (j * 4 + hp * 2) * DKV
            wo[(j * 2 + hp) * 128 : (j * 2 + hp + 1) * 128] = Wo[r0 : r0 + 128]

    f8 = ml_dtypes.float8_e4m3fn

    def to_f8(a, scale):
        return np.clip(a * scale, -224.0, 224.0).astype(f8)

    wo = to_f8(wo, 8.0)
    in_maps = []
    for c in range(8):
        b, g = c // 4, c % 4
        xT = to_f8(np.ascontiguousarray(x[b].T), 16.0)
        xres = np.ascontiguousarray(x[b, g * QS : (g + 1) * QS]).astype(np.float32)
        cols = slice(g * PD, (g + 1) * PD)
        in_maps.append(
            {
                "xT": xT,
                "xres": xres,
                "wq": to_f8(np.ascontiguousarray(Wq[:, cols]), 256.0),
                "wk": to_f8(np.ascontiguousarray(Wk[:, cols]), 256.0),
                "wv": to_f8(np.ascontiguousarray(Wv[:, cols]), 16.0),
                "wo": wo,
                "gamma": np.ascontiguousarray(ln_gamma[None, :]).astype(np.float32),
                "beta": np.ascontiguousarray(ln_beta[None, :]).astype(np.float32),
            }
        )
    return in_maps


def assemble(results, L, D):
    QS = L // 4
    out = np.zeros((2, L, D), np.float32)
    for c in range(8):
        b, g = c // 4, c % 4
        out[b, g * QS : (g + 1) * QS] = results[c]["out"].astype(np.float32)
    return out


def run(x, Wq, Wk, Wv, Wo, ln_gamma, ln_beta, trace=False):
    B, L, D = x.shape
    nc = build_nc(
        L=L, D=D,
        trivial_gamma=bool(np.all(ln_gamma == 1.0)),
        trivial_beta=bool(np.all(ln_beta == 0.0)),
    )
    in_maps = make_in_maps(x, Wq, Wk, Wv, Wo, ln_gamma, ln_beta, L, D)
    res = bass_utils.run_bass_kernel_spmd(
        nc, in_maps, core_ids=list(range(8)), trace=trace
    )
    return assemble(res.results, L, D), res

def kernel(x, Wq, Wk, Wv, Wo, ln_gamma, ln_beta):
    out, _ = run(
        np.asarray(x, np.float32),
        np.asarray(Wq, np.float32),
        np.asarray(Wk, np.float32),
        np.asarray(Wv, np.float32),
        np.asarray(Wo, np.float32),
        np.asarray(ln_gamma, np.float32),
        np.asarray(ln_beta, np.float32),
    )
    return out
